# revision 52
# baseline (speedup 1.0000x reference)
"""DiceLoss kernel for Trainium2 (8 NeuronCores, SPMD data-parallel).

Problem: input [2,4,128,160,160] f32 logits, target [2,128,160,160] int
  pred = argmax(input, axis=1); for classes 1..3 compute
  inter_c = |pred==c & tgt==c|, union_c = |pred==c| + |tgt==c| - inter_c
  loss = 1 - mean_{b,c}( (inter+eps)/(union+eps) )

Sharding: flatten spatial dims (N=3,276,800 voxels per batch); each of the
8 cores gets a contiguous 1/8 slice (S=409,600 voxels) of BOTH batches and
computes per-(batch, class) partial counts; the host sums the 8 tiny
partial-count tensors and finishes the scalar dice math.

The logits are downcast to fp16 on the host before DMA (halves HBM traffic
and doubles DVE throughput). fp16 rounding can tie two classes at the max;
is_equal then counts both (union grows slightly). Measured on the real
input distribution: 1532 tied voxels of 6.5M -> rel err 2.4e-5, far below
tolerance. The target (values 0..3) is exact in fp16.

Engine split per [128, F] chunk (engine-balance, measured modes):
  DVE : tm_c = tensor_scalar(t == c) -> bf16   (4x mode, ~0.26 cyc/elem)
        m01/m23/m = tensor_tensor max (f16, 2x)
        pm_c = tensor_tensor is_equal(x_c, m) -> bf16 (2x)
        (fused accum_out variants force 1x mode - measured 2x slower than
        splitting the reductions onto ACT/PE; plain TT/TS only)
  PE  : inter_c = trace(PM_c^T @ TM_c), PSUM-accumulated per batch
        pred_cnt_c = ones^T @ PM_c column sums, PSUM [1,512] accumulated
        sum(t) = ones^T @ t for the T2 recovery
  ACT : tm1 = relu(1-(t-1)^2), tm3 = relu(t-2) (exact one-hots for
        integer t, one chain each), with accum_out giving the target
        counts T1/T3 for free; T2 solves from the PE sum-of-t on the host.
  Host: traces + count sums + dice (tiny).

(GpSimd compute and tensor_tensor_reduce fail this toolchain's walrus
codegen - avoid.)
"""

import sys

sys.path.insert(0, "/opt/trn_rl_repo")

import numpy as np

# ---------------------------------------------------------------------------
# Hardcoded problem geometry
# ---------------------------------------------------------------------------
B = 2
C = 4
N_SP = 128 * 160 * 160        # 3,276,800 voxels per batch
N_CORES = 8
S = N_SP // N_CORES           # 409,600 voxels per core per batch
P = 128
SF = S // P                   # 3200 free elems per partition
# free-dim chunks (multiples of 128; first smaller for pipeline fill)
CHUNKS = [(0, 640), (640, 1280), (1920, 1280)]
EPS = 1e-08

_CACHE = {}


def _build_bass(chunks=None):
    import concourse.tile as tile
    from concourse import bacc, mybir
    from contextlib import ExitStack

    if chunks is None:
        chunks = CHUNKS
    f32 = mybir.dt.float32
    f16 = mybir.dt.float16
    bf16 = mybir.dt.bfloat16
    Alu = mybir.AluOpType
    Act = mybir.ActivationFunctionType

    nc = bacc.Bacc()

    n_chunks = len(chunks)
    # acc col layout: (b*n_chunks + j)*3 + {0: sum t, 1: sum t^2, 2: sum relu(t-2)}
    n_cols = B * n_chunks * 3
    x = nc.declare_dram_parameter("x", [B, C, S], f16, isOutput=False)
    t = nc.declare_dram_parameter("t", [B, S], f16, isOutput=False)
    acc_d = nc.declare_dram_parameter("acc_out", [P, n_cols], f32, isOutput=True)
    # diag_out[b][:, ci*128:(ci+1)*128] = PM_c^T @ TM_c ; trace = inter_c
    diag_d = nc.declare_dram_parameter("diag_out", [B, P, 384], f32, isOutput=True)
    # cnt_out[b, 0, ci*512:(ci+1)*512] = pred-count column sums;
    # cnt_out[b, 0, 1536:2048] = sum-of-t column sums
    cnt_d = nc.declare_dram_parameter("cnt_out", [B, 1, 2048], f32, isOutput=True)

    with ExitStack() as ctx:
        tc = ctx.enter_context(tile.TileContext(nc))
        const_pool = ctx.enter_context(tc.tile_pool(name="const", bufs=1))
        xpool = ctx.enter_context(tc.tile_pool(name="xp", bufs=3))
        tpool = ctx.enter_context(tc.tile_pool(name="tp", bufs=3))
        mpool = ctx.enter_context(tc.tile_pool(name="mp", bufs=2))
        kpool = ctx.enter_context(tc.tile_pool(name="kp", bufs=6))
        jpool = ctx.enter_context(tc.tile_pool(name="jp", bufs=2))
        opool = ctx.enter_context(tc.tile_pool(name="op", bufs=1))
        pspool = ctx.enter_context(tc.tile_pool(name="ps", bufs=1, space="PSUM"))

        acc = const_pool.tile([P, n_cols], f32)
        ones = const_pool.tile([P, 1], bf16)
        nc.vector.memset(ones[:], 1.0)
        ones16 = const_pool.tile([P, 1], f16, tag="ones16", name="ones16")
        nc.vector.memset(ones16[:], 1.0)
        bias_n2 = const_pool.tile([P, 1], f32, tag="biasn2", name="bias_n2")
        nc.vector.memset(bias_n2[:], -2.0)
        bias_n1 = const_pool.tile([P, 1], f32, tag="biasn1", name="bias_n1")
        nc.vector.memset(bias_n1[:], -1.0)

        # one PSUM bank per class, shared across batches (start=True clears
        # the whole bank, so accumulation groups must not share a bank; with
        # only 8 banks the tiles are reused across b, drained between batches)
        ps_diags = {
            ci: pspool.tile([P, 128], f32, tag=f"diag{ci}", name=f"ps_diag{ci}")
            for ci in range(3)
        }
        ps_cnts = {
            ci: pspool.tile([1, 512], f32, tag=f"cnt{ci}", name=f"ps_cnt{ci}")
            for ci in range(3)
        }
        ps_tcnt = pspool.tile([1, 512], f32, tag="tcnt", name="ps_tcnt")

        for b in range(B):
            for j, (f0, F) in enumerate(chunks):
                tt = tpool.tile([P, F], f16, tag="tt")
                tsrc = t[b, :].rearrange("(p f) -> p f", p=P)
                nc.sync.dma_start(out=tt[:], in_=tsrc[:, f0 : f0 + F])
                xts = []
                for ci in range(C):
                    xc = xpool.tile([P, F], f16, tag=f"x{ci}", name=f"x{ci}")
                    xsrc = x[b, ci, :].rearrange("(p f) -> p f", p=P)
                    nc.sync.dma_start(out=xc[:], in_=xsrc[:, f0 : f0 + F])
                    xts.append(xc)

                # target one-hots: tm2 on DVE (tensor_scalar 4x); tm1 and tm3
                # on ACT - for integer t, relu(t-2) == [t==3] and
                # relu(1-(t-1)^2) == [t==1]; their accum_outs are the target
                # counts T1/T3 directly (T2 solves from sum t via PE tcnt)
                col = (b * n_chunks + j) * 3
                tm2 = kpool.tile([P, F], bf16, tag="tm2", name="tm2")
                nc.vector.tensor_scalar(tm2[:], tt[:], 2.0, None, op0=Alu.is_equal)
                sq1 = jpool.tile([P, F], bf16, tag="sq1", name="sq1")
                nc.scalar.activation(
                    sq1[:], tt[:], Act.Square, bias=bias_n1[:], scale=1.0
                )
                tm1 = kpool.tile([P, F], bf16, tag="tm1", name="tm1")
                nc.scalar.activation(
                    tm1[:], sq1[:], Act.Relu, bias=1.0, scale=-1.0,
                    accum_out=acc[:, col + 1 : col + 2],
                )
                tm3 = kpool.tile([P, F], bf16, tag="tm3", name="tm3")
                nc.scalar.activation(
                    tm3[:], tt[:], Act.Relu, bias=bias_n2[:], scale=1.0,
                    accum_out=acc[:, col + 2 : col + 3],
                )
                tms = [tm1, tm2, tm3]

                m01 = mpool.tile([P, F], f16, tag="m01")
                nc.vector.tensor_tensor(m01[:], xts[0][:], xts[1][:], op=Alu.max)
                m23 = mpool.tile([P, F], f16, tag="m23")
                nc.vector.tensor_tensor(m23[:], xts[2][:], xts[3][:], op=Alu.max)
                m = mpool.tile([P, F], f16, tag="m")
                nc.vector.tensor_tensor(m[:], m01[:], m23[:], op=Alu.max)

                pms = []
                for ci, c in enumerate((1, 2, 3)):
                    pm = kpool.tile([P, F], bf16, tag=f"pm{c}", name=f"pm{c}")
                    nc.vector.tensor_tensor(
                        pm[:], xts[c][:], m[:], op=Alu.is_equal
                    )
                    pms.append(pm)

                last_j = j == n_chunks - 1
                nblk = F // 128
                offs = []
                off = 0
                while off < F:
                    offs.append((off, min(512, F - off)))
                    off += 512
                # sum(t) via ones^T @ t (f16); first in PE order so its stop
                # lands early
                for oi, (off, ns) in enumerate(offs):
                    nc.tensor.matmul(
                        ps_tcnt[0:1, 0:ns],
                        ones16[:],
                        tt[:, off : off + ns],
                        start=(j == 0 and oi == 0),
                        stop=(last_j and oi == len(offs) - 1),
                    )
                # pred counts: ones^T @ PM_c 512-col streams (before the diag
                # matmuls so the batch-end count drain overlaps the diag tail)
                for ci in range(3):
                    for oi, (off, ns) in enumerate(offs):
                        nc.tensor.matmul(
                            ps_cnts[ci][0:1, 0:ns],
                            ones[:],
                            pms[ci][:, off : off + ns],
                            start=(j == 0 and oi == 0),
                            stop=(last_j and oi == len(offs) - 1),
                        )
                # inter_c: trace-matmul accumulation per 128-col block
                for si in range(nblk):
                    sl = slice(si * 128, (si + 1) * 128)
                    for ci in range(3):
                        nc.tensor.matmul(
                            ps_diags[ci][:, :],
                            pms[ci][:, sl],
                            tms[ci][:, sl],
                            start=(j == 0 and si == 0),
                            stop=(last_j and si == nblk - 1),
                        )

            # drain PSUM -> SBUF (DMA cannot read PSUM), then DMA out
            sb_diag = opool.tile([P, 384], f32, tag=f"sbd{b}", name=f"sb_diag{b}")
            for ci in range(3):
                nc.scalar.copy(
                    sb_diag[:, ci * 128 : (ci + 1) * 128], ps_diags[ci][:]
                )
            nc.sync.dma_start(out=diag_d[b, :, :], in_=sb_diag[:])
            sb_cnt = opool.tile([1, 2048], f32, tag=f"sbc{b}", name=f"sb_cnt{b}")
            for ci in range(3):
                nc.vector.tensor_copy(
                    sb_cnt[0:1, ci * 512 : (ci + 1) * 512], ps_cnts[ci][0:1, :]
                )
            nc.vector.tensor_copy(sb_cnt[0:1, 1536:2048], ps_tcnt[0:1, :])
            nc.sync.dma_start(out=cnt_d[b, :, :], in_=sb_cnt[0:1, :])

        nc.sync.dma_start(out=acc_d[:, :], in_=acc[:])

    nc.compile()
    return nc


def _get_nc():
    if "nc" not in _CACHE:
        _CACHE["nc"] = _build_bass()
    return _CACHE["nc"]


def _shard_inputs(input, target):
    inp = np.asarray(input, dtype=np.float32).reshape(B, C, N_SP)
    inp16 = inp.astype(np.float16)
    tgt16 = np.asarray(target).reshape(B, N_SP).astype(np.float16)
    in_maps = []
    for r in range(N_CORES):
        xr = np.ascontiguousarray(inp16[:, :, r * S : (r + 1) * S])
        tr = np.ascontiguousarray(tgt16[:, r * S : (r + 1) * S])
        in_maps.append({"x": xr, "t": tr})
    return in_maps


def _finish(results):
    """Combine per-core partial counts into the dice loss."""
    inter = np.zeros((B, 3), np.float64)
    pred_cnt = np.zeros((B, 3), np.float64)
    tgt_cnt = np.zeros((B, 3), np.float64)
    n_chunks = len(CHUNKS)
    for res in results:
        acc = np.asarray(res["acc_out"], np.float64)    # [128, n_cols]
        diag = np.asarray(res["diag_out"], np.float64)  # [B, 128, 384]
        cnt = np.asarray(res["cnt_out"], np.float64)    # [B, 1, 1536]
        for b in range(B):
            s1 = cnt[b, 0, 1536:2048].sum()    # sum t
            s2 = s3 = 0.0                      # T1, T3 partial sums
            for j in range(n_chunks):
                col = (b * n_chunks + j) * 3
                s2 += acc[:, col + 1].sum()
                s3 += acc[:, col + 2].sum()
            t1 = s2
            t3 = s3
            t2 = (s1 - t1 - 3.0 * t3) / 2.0
            tgt_cnt[b, 0] += t1
            tgt_cnt[b, 1] += t2
            tgt_cnt[b, 2] += t3
            for ci in range(3):
                blk = diag[b][:, ci * 128 : (ci + 1) * 128]
                inter[b, ci] += np.trace(blk)
                pred_cnt[b, ci] += cnt[b, 0, ci * 512 : (ci + 1) * 512].sum()
    union = pred_cnt + tgt_cnt - inter
    dice = (inter + EPS) / (union + EPS)
    return np.float32(1.0 - dice.mean())


def kernel(input, target):
    from concourse.bass_utils import run_bass_kernel_spmd

    nc = _get_nc()
    in_maps = _shard_inputs(input, target)
    out = run_bass_kernel_spmd(nc, in_maps, core_ids=list(range(N_CORES)))
    return _finish(out.results)


if __name__ == "__main__":
    # Smoke test with random data against a numpy reference.
    rng = np.random.default_rng(0)
    inp = rng.standard_normal((B, C, 128, 160, 160), dtype=np.float32)
    tgt = rng.integers(0, C, size=(B, 128, 160, 160)).astype(np.int32)

    got = kernel(input=inp, target=tgt)

    pred = np.argmax(inp, axis=1).reshape(B, -1)
    tg = tgt.reshape(B, -1)
    dice = np.zeros((B, 3))
    for b in range(B):
        for ci, c in enumerate((1, 2, 3)):
            pm = pred[b] == c
            tm = tg[b] == c
            i = np.sum(pm & tm)
            u = np.sum(pm | tm)
            dice[b, ci] = (i + EPS) / (u + EPS)
    want = np.float32(1.0 - dice.mean())
    print("kernel:", got, "reference:", want, "relerr:", abs(got - want) / abs(want))


# revision 55
# speedup vs baseline: 1.0081x; 1.0081x over previous
"""DiceLoss kernel for Trainium2 (8 NeuronCores, SPMD data-parallel).

Problem: input [2,4,128,160,160] f32 logits, target [2,128,160,160] int
  pred = argmax(input, axis=1); for classes 1..3 compute
  inter_c = |pred==c & tgt==c|, union_c = |pred==c| + |tgt==c| - inter_c
  loss = 1 - mean_{b,c}( (inter+eps)/(union+eps) )

Sharding: flatten spatial dims (N=3,276,800 voxels per batch); each of the
8 cores gets a contiguous 1/8 slice (S=409,600 voxels) of BOTH batches and
computes per-(batch, class) partial counts; the host sums the 8 tiny
partial-count tensors and finishes the scalar dice math.

The logits are downcast to fp16 on the host before DMA (halves HBM traffic
and doubles DVE throughput). fp16 rounding can tie two classes at the max;
is_equal then counts both (union grows slightly). Measured on the real
input distribution: 1532 tied voxels of 6.5M -> rel err 2.4e-5, far below
tolerance. The target (values 0..3) is exact in fp16.

Engine split per [128, F] chunk (engine-balance, measured modes):
  DVE : tm_c = tensor_scalar(t == c) -> bf16   (4x mode, ~0.26 cyc/elem)
        m01/m23/m = tensor_tensor max (f16, 2x)
        pm_c = tensor_tensor is_equal(x_c, m) -> bf16 (2x)
        (fused accum_out variants force 1x mode - measured 2x slower than
        splitting the reductions onto ACT/PE; plain TT/TS only)
  PE  : inter_c = trace(PM_c^T @ TM_c), PSUM-accumulated per batch
        pred_cnt_c = ones^T @ PM_c column sums, PSUM [1,512] accumulated
        sum(t) = ones^T @ t for the T2 recovery
  ACT : tm1 = relu(1-(t-1)^2), tm3 = relu(t-2) (exact one-hots for
        integer t, one chain each), with accum_out giving the target
        counts T1/T3 for free; T2 solves from the PE sum-of-t on the host.
  Host: traces + count sums + dice (tiny).

(GpSimd compute and tensor_tensor_reduce fail this toolchain's walrus
codegen - avoid.)
"""

import sys

sys.path.insert(0, "/opt/trn_rl_repo")

import numpy as np

# ---------------------------------------------------------------------------
# Hardcoded problem geometry
# ---------------------------------------------------------------------------
B = 2
C = 4
N_SP = 128 * 160 * 160        # 3,276,800 voxels per batch
N_CORES = 8
S = N_SP // N_CORES           # 409,600 voxels per core per batch
P = 128
SF = S // P                   # 3200 free elems per partition
# free-dim chunks (multiples of 128; first smaller for pipeline fill)
CHUNKS = [(0, 640), (640, 1280), (1920, 1280)]
EPS = 1e-08

_CACHE = {}


def _build_bass(chunks=None):
    import concourse.tile as tile
    from concourse import bacc, mybir
    from contextlib import ExitStack

    if chunks is None:
        chunks = CHUNKS
    f32 = mybir.dt.float32
    f16 = mybir.dt.float16
    bf16 = mybir.dt.bfloat16
    Alu = mybir.AluOpType
    Act = mybir.ActivationFunctionType

    nc = bacc.Bacc()

    n_chunks = len(chunks)
    # acc col layout: (b*n_chunks + j)*3 + {0: sum t, 1: sum t^2, 2: sum relu(t-2)}
    n_cols = B * n_chunks * 3
    x = nc.declare_dram_parameter("x", [B, C, S], f16, isOutput=False)
    t = nc.declare_dram_parameter("t", [B, S], f16, isOutput=False)
    acc_d = nc.declare_dram_parameter("acc_out", [P, n_cols], f32, isOutput=True)
    # diag_out[b][:, ci*128:(ci+1)*128] = PM_c^T @ TM_c ; trace = inter_c
    diag_d = nc.declare_dram_parameter("diag_out", [B, P, 384], f32, isOutput=True)
    # cnt_out[b, 0, ci*512:(ci+1)*512] = pred-count column sums;
    # cnt_out[b, 0, 1536:2048] = sum-of-t column sums
    cnt_d = nc.declare_dram_parameter("cnt_out", [B, 1, 2048], f32, isOutput=True)

    with ExitStack() as ctx:
        tc = ctx.enter_context(tile.TileContext(nc))
        const_pool = ctx.enter_context(tc.tile_pool(name="const", bufs=1))
        xpool = ctx.enter_context(tc.tile_pool(name="xp", bufs=3))
        tpool = ctx.enter_context(tc.tile_pool(name="tp", bufs=3))
        mpool = ctx.enter_context(tc.tile_pool(name="mp", bufs=2))
        kpool = ctx.enter_context(tc.tile_pool(name="kp", bufs=6))
        jpool = ctx.enter_context(tc.tile_pool(name="jp", bufs=2))
        opool = ctx.enter_context(tc.tile_pool(name="op", bufs=1))
        pspool = ctx.enter_context(tc.tile_pool(name="ps", bufs=1, space="PSUM"))

        acc = const_pool.tile([P, n_cols], f32)
        ones = const_pool.tile([P, 1], bf16)
        nc.vector.memset(ones[:], 1.0)
        ones16 = const_pool.tile([P, 1], f16, tag="ones16", name="ones16")
        nc.vector.memset(ones16[:], 1.0)
        bias_n2 = const_pool.tile([P, 1], f32, tag="biasn2", name="bias_n2")
        nc.vector.memset(bias_n2[:], -2.0)
        bias_n1 = const_pool.tile([P, 1], f32, tag="biasn1", name="bias_n1")
        nc.vector.memset(bias_n1[:], -1.0)

        # one PSUM bank per class, shared across batches (start=True clears
        # the whole bank, so accumulation groups must not share a bank; with
        # only 8 banks the tiles are reused across b, drained between batches)
        ps_diags = {
            ci: pspool.tile([P, 128], f32, tag=f"diag{ci}", name=f"ps_diag{ci}")
            for ci in range(3)
        }
        ps_cnts = {
            ci: pspool.tile([1, 512], f32, tag=f"cnt{ci}", name=f"ps_cnt{ci}")
            for ci in range(3)
        }
        ps_tcnt = pspool.tile([1, 512], f32, tag="tcnt", name="ps_tcnt")

        for b in range(B):
            for j, (f0, F) in enumerate(chunks):
                tt = tpool.tile([P, F], f16, tag="tt")
                tsrc = t[b, :].rearrange("(p f) -> p f", p=P)
                nc.sync.dma_start(out=tt[:], in_=tsrc[:, f0 : f0 + F])
                xts = []
                for ci in range(C):
                    xc = xpool.tile([P, F], f16, tag=f"x{ci}", name=f"x{ci}")
                    xsrc = x[b, ci, :].rearrange("(p f) -> p f", p=P)
                    nc.sync.dma_start(out=xc[:], in_=xsrc[:, f0 : f0 + F])
                    xts.append(xc)

                # target one-hots: tm2 on DVE (tensor_scalar 4x); tm1 and tm3
                # on ACT - for integer t, relu(t-2) == [t==3] and
                # relu(1-(t-1)^2) == [t==1]; their accum_outs are the target
                # counts T1/T3 directly (T2 solves from sum t via PE tcnt)
                col = (b * n_chunks + j) * 3
                tm2 = kpool.tile([P, F], bf16, tag="tm2", name="tm2")
                nc.vector.tensor_scalar(tm2[:], tt[:], 2.0, None, op0=Alu.is_equal)
                sq1 = jpool.tile([P, F], bf16, tag="sq1", name="sq1")
                nc.scalar.activation(
                    sq1[:], tt[:], Act.Square, bias=bias_n1[:], scale=1.0
                )
                tm1 = kpool.tile([P, F], bf16, tag="tm1", name="tm1")
                nc.scalar.activation(
                    tm1[:], sq1[:], Act.Relu, bias=1.0, scale=-1.0,
                    accum_out=acc[:, col + 1 : col + 2],
                )
                tm3 = kpool.tile([P, F], bf16, tag="tm3", name="tm3")
                nc.scalar.activation(
                    tm3[:], tt[:], Act.Relu, bias=bias_n2[:], scale=1.0,
                    accum_out=acc[:, col + 2 : col + 3],
                )
                tms = [tm1, tm2, tm3]

                m01 = mpool.tile([P, F], f16, tag="m01")
                nc.vector.tensor_tensor(m01[:], xts[0][:], xts[1][:], op=Alu.max)
                m23 = mpool.tile([P, F], f16, tag="m23")
                nc.vector.tensor_tensor(m23[:], xts[2][:], xts[3][:], op=Alu.max)
                m = mpool.tile([P, F], f16, tag="m")
                nc.vector.tensor_tensor(m[:], m01[:], m23[:], op=Alu.max)

                pms = []
                for ci, c in enumerate((1, 2, 3)):
                    pm = kpool.tile([P, F], bf16, tag=f"pm{c}", name=f"pm{c}")
                    nc.vector.tensor_tensor(
                        pm[:], xts[c][:], m[:], op=Alu.is_equal
                    )
                    pms.append(pm)

                last_j = j == n_chunks - 1
                nblk = F // 128
                offs = []
                off = 0
                while off < F:
                    offs.append((off, min(512, F - off)))
                    off += 512
                # sum(t) via ones^T @ t (f16); first in PE order so its stop
                # lands early
                for oi, (off, ns) in enumerate(offs):
                    nc.tensor.matmul(
                        ps_tcnt[0:1, 0:ns],
                        ones16[:],
                        tt[:, off : off + ns],
                        start=(j == 0 and oi == 0),
                        stop=(last_j and oi == len(offs) - 1),
                    )
                # pred counts: ones^T @ PM_c 512-col streams (before the diag
                # matmuls so the batch-end count drain overlaps the diag tail)
                for ci in range(3):
                    for oi, (off, ns) in enumerate(offs):
                        nc.tensor.matmul(
                            ps_cnts[ci][0:1, 0:ns],
                            ones[:],
                            pms[ci][:, off : off + ns],
                            start=(j == 0 and oi == 0),
                            stop=(last_j and oi == len(offs) - 1),
                        )
                # inter_c: trace-matmul accumulation per 128-col block
                for si in range(nblk):
                    sl = slice(si * 128, (si + 1) * 128)
                    for ci in range(3):
                        nc.tensor.matmul(
                            ps_diags[ci][:, :],
                            pms[ci][:, sl],
                            tms[ci][:, sl],
                            start=(j == 0 and si == 0),
                            stop=(last_j and si == nblk - 1),
                        )

            # drain PSUM -> SBUF (DMA cannot read PSUM), then DMA out
            sb_diag = opool.tile([P, 384], f32, tag=f"sbd{b}", name=f"sb_diag{b}")
            for ci in range(3):
                nc.scalar.copy(
                    sb_diag[:, ci * 128 : (ci + 1) * 128], ps_diags[ci][:]
                )
            nc.sync.dma_start(out=diag_d[b, :, :], in_=sb_diag[:])
            sb_cnt = opool.tile([1, 2048], f32, tag=f"sbc{b}", name=f"sb_cnt{b}")
            for ci in range(3):
                nc.vector.tensor_copy(
                    sb_cnt[0:1, ci * 512 : (ci + 1) * 512], ps_cnts[ci][0:1, :]
                )
            nc.vector.tensor_copy(sb_cnt[0:1, 1536:2048], ps_tcnt[0:1, :])
            nc.sync.dma_start(out=cnt_d[b, :, :], in_=sb_cnt[0:1, :])

        nc.sync.dma_start(out=acc_d[:, :], in_=acc[:])

    nc.compile()
    return nc


def _get_nc():
    if "nc" not in _CACHE:
        _CACHE["nc"] = _build_bass()
    return _CACHE["nc"]


def _shard_inputs(input, target):
    inp = np.asarray(input, dtype=np.float32).reshape(B, C, N_SP)
    inp16 = inp.astype(np.float16)
    tgt16 = np.asarray(target).reshape(B, N_SP).astype(np.float16)
    in_maps = []
    for r in range(N_CORES):
        xr = np.ascontiguousarray(inp16[:, :, r * S : (r + 1) * S])
        tr = np.ascontiguousarray(tgt16[:, r * S : (r + 1) * S])
        in_maps.append({"x": xr, "t": tr})
    return in_maps


def _finish(results):
    """Combine per-core partial counts into the dice loss."""
    inter = np.zeros((B, 3), np.float64)
    pred_cnt = np.zeros((B, 3), np.float64)
    tgt_cnt = np.zeros((B, 3), np.float64)
    n_chunks = len(CHUNKS)
    for res in results:
        acc = np.asarray(res["acc_out"], np.float64)    # [128, n_cols]
        diag = np.asarray(res["diag_out"], np.float64)  # [B, 128, 384]
        cnt = np.asarray(res["cnt_out"], np.float64)    # [B, 1, 1536]
        for b in range(B):
            s1 = cnt[b, 0, 1536:2048].sum()    # sum t
            s2 = s3 = 0.0                      # T1, T3 partial sums
            for j in range(n_chunks):
                col = (b * n_chunks + j) * 3
                s2 += acc[:, col + 1].sum()
                s3 += acc[:, col + 2].sum()
            t1 = s2
            t3 = s3
            t2 = (s1 - t1 - 3.0 * t3) / 2.0
            tgt_cnt[b, 0] += t1
            tgt_cnt[b, 1] += t2
            tgt_cnt[b, 2] += t3
            for ci in range(3):
                blk = diag[b][:, ci * 128 : (ci + 1) * 128]
                inter[b, ci] += np.trace(blk)
                pred_cnt[b, ci] += cnt[b, 0, ci * 512 : (ci + 1) * 512].sum()
    union = pred_cnt + tgt_cnt - inter
    dice = (inter + EPS) / (union + EPS)
    return np.float32(1.0 - dice.mean())


def kernel(input, target):
    from concourse.bass_utils import run_bass_kernel_spmd

    nc = _get_nc()
    in_maps = _shard_inputs(input, target)
    out = run_bass_kernel_spmd(nc, in_maps, core_ids=list(range(N_CORES)))
    return _finish(out.results)


if __name__ == "__main__":
    # Smoke test with random data against a numpy reference.
    rng = np.random.default_rng(0)
    inp = rng.standard_normal((B, C, 128, 160, 160), dtype=np.float32)
    tgt = rng.integers(0, C, size=(B, 128, 160, 160)).astype(np.int32)

    got = kernel(input=inp, target=tgt)

    pred = np.argmax(inp, axis=1).reshape(B, -1)
    tg = tgt.reshape(B, -1)
    dice = np.zeros((B, 3))
    for b in range(B):
        for ci, c in enumerate((1, 2, 3)):
            pm = pred[b] == c
            tm = tg[b] == c
            i = np.sum(pm & tm)
            u = np.sum(pm | tm)
            dice[b, ci] = (i + EPS) / (u + EPS)
    want = np.float32(1.0 - dice.mean())
    print("kernel:", got, "reference:", want, "relerr:", abs(got - want) / abs(want))
